# revision 30
# baseline (speedup 1.0000x reference)
"""Trainium2 Bass kernel: 31x31 valid cross-correlation of a 4096x4096 fp32
image, plus scalar bias.

Strategy (per NeuronCore):
  - Image is split into 8 row bands (512 output rows per core) with 30-row
    halos; weight/bias replicated.
  - On each core the conv is computed as a sum over the 31 kernel *columns* b
    of banded (Toeplitz) matmuls on the PE array:
        out[m, n] = sum_b sum_k Wt[k, b, m] * x[r0 + k, c0 + b + n]
    where Wt[k, b, m] = weight[k - m, b] (0 <= k-m < 31).  The moving operand
    is the image tile in its natural row-major layout (partition = image row),
    shifted by b along the free dim; the stationary operand is a [128, 98]
    Toeplitz band built on the host from the 31x31 weight.  31 matmuls
    accumulate into one PSUM bank; ScalarE evicts PSUM -> SBUF fusing the
    bias add; DMA writes the band back to DRAM.
"""

import sys
import types

import numpy as np

# Cache compiled NEFFs across processes (keyed by HLO bytes, so stale hits
# are impossible); also makes repeat runs fast.
try:
    import jax

    jax.config.update("jax_compilation_cache_dir", "/tmp/jax_comp_cache")
    jax.config.update("jax_persistent_cache_min_entry_size_bytes", 0)
    jax.config.update("jax_persistent_cache_min_compile_time_secs", 0.0)
except Exception:
    pass

# This axon client has no antenv.axon_hooks; stub it so
# run_bass_kernel_spmd(trace=True) degrades to no-trace instead of crashing.
try:
    import antenv.axon_hooks  # noqa: F401
except ImportError:
    import antenv

    _stub = types.ModuleType("antenv.axon_hooks")
    _stub.get_axon_ntff_profile_hook = lambda: None
    sys.modules["antenv.axon_hooks"] = _stub
    antenv.axon_hooks = _stub

import concourse.bass as bass
import concourse.bacc as bacc
import concourse.mybir as mybir
import concourse.tile as tile
from concourse.bass_utils import run_bass_kernel_spmd

H = 4096
W = 4096
KH = 31
KW = 31
OH = H - KH + 1  # 4066
OW = W - KW + 1  # 4066
N_CORES = 8
ROWS_PER_CORE = 512          # output rows computed per core (last core: tail junk)
IN_ROWS = ROWS_PER_CORE + KH - 1  # 542

M_TILE = 98                  # 128 - 30: output rows per PSUM tile
N_TILE = 512                 # one PSUM bank of fp32

F32 = mybir.dt.float32
F32R = mybir.dt.float32r


def build_program(rows_out, in_rows, width, kh=KH, kw=KW, use_f32r=True,
                  repeat=1):
    """Build the per-core Bass program (identical on all cores)."""
    out_w = width - kw + 1
    mm_dt = F32R if use_f32r else F32
    nc = bacc.Bacc(None, target_bir_lowering=False, debug=False)

    x_d = nc.declare_dram_parameter("x", [in_rows, width], F32, isOutput=False)
    wt_d = nc.declare_dram_parameter("wt", [128, kw, M_TILE], F32, isOutput=False)
    bias_d = nc.declare_dram_parameter("bias", [1], F32, isOutput=False)
    out_d = nc.declare_dram_parameter("out", [rows_out, out_w], F32, isOutput=True)

    m_tiles = []
    r0 = 0
    while r0 < rows_out:
        m = min(M_TILE, rows_out - r0)
        k = min(128, in_rows - r0)
        m_tiles.append((r0, m, k))
        r0 += m
    n_tiles = []
    c0 = 0
    while c0 < out_w:
        n = min(N_TILE, out_w - c0)
        n_tiles.append((c0, n))
        c0 += n

    with tile.TileContext(nc) as tc:
        with (
            tc.tile_pool(name="const", bufs=1) as cpool,
            tc.tile_pool(name="xin", bufs=2) as xpool,
            tc.tile_pool(name="oev", bufs=4) as opool,
            tc.tile_pool(name="ps", bufs=4, space="PSUM") as pspool,
        ):
            wt = cpool.tile([128, kw, M_TILE], mm_dt)
            nc.sync.dma_start(wt[:], wt_d.ap().bitcast(mm_dt))
            bias_t = cpool.tile([128, 1], F32)
            nc.sync.dma_start(bias_t[:], bias_d.ap().to_broadcast((128, 1)))

            import contextlib

            rep_ctx = (
                tc.For_i(0, repeat, 1) if repeat > 1 else contextlib.nullcontext()
            )
            with rep_ctx:
                for (r0, m, k) in m_tiles:
                    xt = xpool.tile([128, width], mm_dt)
                    nc.sync.dma_start(
                        xt[:k, :], x_d[r0 : r0 + k, :].bitcast(mm_dt)
                    )
                    for (c0, n) in n_tiles:
                        ps = pspool.tile([M_TILE, N_TILE], F32)
                        for b in range(kw):
                            nc.tensor.matmul(
                                ps[:m, :n],
                                wt[:k, b, :m],
                                xt[:k, c0 + b : c0 + b + n],
                                start=(b == 0),
                                stop=(b == kw - 1),
                            )
                        ot = opool.tile([M_TILE, N_TILE], F32)
                        nc.scalar.activation(
                            ot[:m, :n],
                            ps[:m, :n],
                            mybir.ActivationFunctionType.Identity,
                            bias=bias_t[:m],
                        )
                        nc.sync.dma_start(
                            out_d[r0 : r0 + m, c0 : c0 + n], ot[:m, :n]
                        )

    nc.compile()
    return nc


def build_program_g3(rows_out, in_rows, width, kh=KH, kw=KW, repeat=1,
                     mm_dtype=None, use_f32r=True):
    """3-pass Dekker-split variant: the matmul dtype keeps p mantissa bits
    (fp32r: 11 truncated; fp16: 10 rounded), so conv(x, w) is computed as
      conv(xh, wh) + conv(xh, wl) + conv(xl, wh)
    with xh = round_p(x), xl = x - xh (likewise w), all three passes
    accumulating into the same PSUM tiles.  Same sheared col-tiled structure
    as build_program_g, one PSUM bank per output row sub-block q."""
    assert rows_out % 128 == 0
    out_w = width - kw + 1
    sw = out_w + kh - 1
    xw = width + 1
    n_groups = rows_out // 128
    n_u = (kw + 1) // 2
    if mm_dtype is not None:
        mm_dt = mm_dtype
    else:
        mm_dt = F32R if use_f32r else F32
    if mm_dt == mybir.dt.bfloat16:
        io_dt = mybir.dt.bfloat16
    elif mm_dt == mybir.dt.float16:
        io_dt = mybir.dt.float16
    else:
        io_dt = F32

    nc = bacc.Bacc(None, target_bir_lowering=False, debug=False)
    xh_d = nc.declare_dram_parameter("xh", [in_rows, xw], io_dt, isOutput=False)
    xl_d = nc.declare_dram_parameter("xl", [in_rows, xw], io_dt, isOutput=False)
    wt_d = nc.declare_dram_parameter("wt", [2, 128, n_u, 32], io_dt, isOutput=False)
    bias_d = nc.declare_dram_parameter("bias", [1], F32, isOutput=False)
    out_d = nc.declare_dram_parameter("out", [rows_out, out_w], F32, isOutput=True)

    n_tiles = []
    c0 = 0
    while c0 < out_w:
        n_tiles.append((c0, min(N_TILE, out_w - c0)))
        c0 += N_TILE

    with tile.TileContext(nc) as tc:
        with (
            tc.tile_pool(name="const", bufs=1) as cpool,
            tc.tile_pool(name="xin", bufs=8) as xpool,
            tc.tile_pool(name="oev", bufs=4) as opool,
            tc.tile_pool(name="ps", bufs=2, space="PSUM") as pspool,
        ):
            wth = cpool.tile([128, n_u, 32], mm_dt)
            nc.sync.dma_start(wth[:], wt_d[0].bitcast(mm_dt))
            wtl = cpool.tile([128, n_u, 32], mm_dt)
            nc.sync.dma_start(wtl[:], wt_d[1].bitcast(mm_dt))
            bias_t = cpool.tile([128, 1], F32)
            nc.sync.dma_start(bias_t[:], bias_d.ap().to_broadcast((128, 1)))

            import contextlib

            rep_ctx = (
                tc.For_i(0, repeat, 1) if repeat > 1 else contextlib.nullcontext()
            )
            with rep_ctx:
                for g in range(n_groups):
                    shs, sls = [], []
                    for q in range(4):
                        r0 = 128 * g + 32 * q
                        sh = xpool.tile([124, sw], mm_dt, name=f"sh{q}", tag="sh")
                        nc.sync.dma_start(
                            sh[:],
                            bass.AP(xh_d, r0 * xw, [[xw, 62], [1, 2], [1, sw]])
                            .bitcast(mm_dt),
                        )
                        shs.append(sh)
                        sl = xpool.tile([124, sw], mm_dt, name=f"sl{q}", tag="sl")
                        nc.sync.dma_start(
                            sl[:],
                            bass.AP(xl_d, r0 * xw, [[xw, 62], [1, 2], [1, sw]])
                            .bitcast(mm_dt),
                        )
                        sls.append(sl)
                    for (c0, n) in n_tiles:
                        pss = [
                            pspool.tile([128, N_TILE], F32, name=f"psq{q}",
                                        tag=f"ps{q}")
                            for q in range(4)
                        ]
                        passes = [(wth, shs), (wtl, shs), (wth, sls)]
                        for pi, (wtp, stp) in enumerate(passes):
                            for u in range(n_u):
                                for q in range(4):
                                    nc.tensor.matmul(
                                        pss[q][32 * q : 32 * q + 32, :n],
                                        wtp[:124, u, :],
                                        stp[q][:, c0 + 2 * u : c0 + 2 * u + n],
                                        start=(pi == 0 and u == 0),
                                        stop=(pi == 2 and u == n_u - 1),
                                        tile_position=(0, 32 * q),
                                    )
                        ot = opool.tile([128, N_TILE], F32)
                        for q in range(4):
                            sl_ = slice(32 * q, 32 * q + 32)
                            if q < 2:
                                nc.scalar.activation(
                                    ot[sl_, :n],
                                    pss[q][sl_, :n],
                                    mybir.ActivationFunctionType.Identity,
                                    bias=bias_t[sl_],
                                )
                            else:
                                nc.vector.tensor_tensor(
                                    ot[sl_, :n],
                                    pss[q][sl_, :n],
                                    bias_t[sl_].to_broadcast((32, n)),
                                    mybir.AluOpType.add,
                                )
                        nc.sync.dma_start(
                            out_d[128 * g : 128 * g + 128, c0 : c0 + n],
                            ot[:, :n],
                        )

    nc.compile()
    return nc


def trunc11(x):
    """Truncate fp32 mantissa to 11 bits (what fp32r matmul does to operands)."""
    u = np.ascontiguousarray(x, dtype=np.float32).view(np.uint32)
    return (u & np.uint32(0xFFFFF000)).view(np.float32)


def build_program_g(rows_out, in_rows, width, kh=KH, kw=KW, use_f32r=True,
                    repeat=1, mm_dtype=None):
    """Sheared 2-shift variant: K=124 = 62 rows x 2 column-shifts (interleaved
    on partitions), M=32 output rows per matmul, 16 accumulating matmuls per
    output tile (u packs kernel columns b=2u and 2u+1), 4 concurrent matmuls
    on distinct PE column-groups (output row sub-blocks q=0..3 of each
    128-row group)."""
    assert rows_out % 128 == 0
    out_w = width - kw + 1
    sw = out_w + kh - 1  # shear tile width (= width - 1... = out_w + 30)
    xw = width + 1       # DRAM x band width (1 zero pad col for shift s=1)
    n_groups = rows_out // 128
    n_u = (kw + 1) // 2  # 16
    if mm_dtype is not None:
        mm_dt = mm_dtype
    else:
        mm_dt = F32R if use_f32r else F32
    if mm_dt == mybir.dt.bfloat16:
        io_dt = mybir.dt.bfloat16
    elif mm_dt == mybir.dt.float16:
        io_dt = mybir.dt.float16
    else:
        io_dt = F32

    nc = bacc.Bacc(None, target_bir_lowering=False, debug=False)
    x_d = nc.declare_dram_parameter("x", [in_rows, xw], io_dt, isOutput=False)
    wt_d = nc.declare_dram_parameter("wt", [128, n_u, 32], io_dt, isOutput=False)
    bias_d = nc.declare_dram_parameter("bias", [1], F32, isOutput=False)
    out_d = nc.declare_dram_parameter("out", [rows_out, out_w], F32, isOutput=True)

    n_tiles = []
    c0 = 0
    while c0 < out_w:
        n_tiles.append((c0, min(N_TILE, out_w - c0)))
        c0 += N_TILE

    with tile.TileContext(nc) as tc:
        with (
            tc.tile_pool(name="const", bufs=1) as cpool,
            tc.tile_pool(name="xin", bufs=8) as xpool,
            tc.tile_pool(name="oev", bufs=4) as opool,
            tc.tile_pool(name="ps", bufs=2, space="PSUM") as pspool,
        ):
            wt = cpool.tile([128, n_u, 32], mm_dt)
            nc.sync.dma_start(wt[:], wt_d.ap().bitcast(mm_dt))
            bias_t = cpool.tile([128, 1], F32)
            nc.sync.dma_start(bias_t[:], bias_d.ap().to_broadcast((128, 1)))

            import contextlib

            rep_ctx = (
                tc.For_i(0, repeat, 1) if repeat > 1 else contextlib.nullcontext()
            )
            with rep_ctx:
                _emit_g_body(
                    nc, tc, xpool, opool, pspool, wt, bias_t,
                    x_d, out_d, n_groups, n_tiles, n_u, sw, xw, mm_dt,
                )

    nc.compile()
    return nc


def _emit_g_body(nc, tc, xpool, opool, pspool, wt, bias_t, x_d, out_d,
                 n_groups, n_tiles, n_u, sw, xw, mm_dt):
    for g in range(n_groups):
        # 4 shear tiles for this 128-row output group
        sts = []
        for q in range(4):
            r0 = 128 * g + 32 * q
            st = xpool.tile([124, sw], mm_dt)
            src = bass.AP(x_d, r0 * xw, [[xw, 62], [1, 2], [1, sw]])
            nc.sync.dma_start(st[:], src.bitcast(mm_dt))
            sts.append(st)
        for (c0, n) in n_tiles:
            # one PSUM bank per q so the 4 accumulation chains have no
            # same-bank deps and stream on disjoint PE subarray columns
            pss = [pspool.tile([128, N_TILE], F32, name=f"psq{q}", tag=f"ps{q}") for q in range(4)]
            for u in range(n_u):
                for q in range(4):
                    nc.tensor.matmul(
                        pss[q][32 * q : 32 * q + 32, :n],
                        wt[:124, u, :],
                        sts[q][:, c0 + 2 * u : c0 + 2 * u + n],
                        start=(u == 0),
                        stop=(u == n_u - 1),
                        tile_position=(0, 32 * q),
                    )
            ot = opool.tile([128, N_TILE], F32)
            for q in range(4):
                sl = slice(32 * q, 32 * q + 32)
                if q < 2:
                    nc.scalar.activation(
                        ot[sl, :n],
                        pss[q][sl, :n],
                        mybir.ActivationFunctionType.Identity,
                        bias=bias_t[sl],
                    )
                else:
                    nc.vector.tensor_tensor(
                        ot[sl, :n],
                        pss[q][sl, :n],
                        bias_t[sl].to_broadcast((32, n)),
                        mybir.AluOpType.add,
                    )
            nc.sync.dma_start(
                out_d[128 * g : 128 * g + 128, c0 : c0 + n], ot[:, :n]
            )


def build_shear_weights(weight, kh=KH, kw=KW):
    """wtg[2r + s, u, m] = weight[r - m, 2u + s] for 0 <= r - m < kh, b < kw."""
    n_u = (kw + 1) // 2
    w = np.asarray(weight, dtype=np.float32)
    wtg = np.zeros((128, n_u, 32), dtype=np.float32)
    for r in range(62):
        for s in range(2):
            for m in range(32):
                a = r - m
                if 0 <= a < kh:
                    for u in range(n_u):
                        b = 2 * u + s
                        if b < kw:
                            wtg[2 * r + s, u, m] = w[a, b]
    return wtg


def build_toeplitz(weight, kh=KH, kw=KW):
    """Wt[k, b, m] = weight[k - m, b] for 0 <= k - m < kh, else 0."""
    wt = np.zeros((128, kw, M_TILE), dtype=np.float32)
    w = np.asarray(weight, dtype=np.float32)
    for a in range(kh):
        for m in range(M_TILE):
            k = a + m
            if k < 128:
                wt[k, :, m] = w[a, :]
    return wt


# 2x4 grid sharding (variant "a2"): 2 row bands x 4 col bands.
GRID_R, GRID_C = 2, 4
ROWS_A2 = 2048            # out rows per core (covers 2033 valid + junk)
IN_ROWS_A2 = ROWS_A2 + KH - 1   # 2078
COLS_A2 = 1018            # out cols per core (4x1018 = 4072 >= 4066; even N tiles for fp32r)
W_A2 = COLS_A2 + KW - 1   # 1048

DEFAULT_VARIANT = "gh"

_PROGRAM_CACHE = {}


def _build_fn_for_variant(variant, use_f32r=True):
    import functools

    if variant == "g":
        return functools.partial(
            build_program_g, ROWS_PER_CORE, IN_ROWS, W, use_f32r=use_f32r
        )
    if variant == "gb":
        return functools.partial(
            build_program_g, ROWS_PER_CORE, IN_ROWS, W,
            mm_dtype=mybir.dt.bfloat16,
        )
    if variant == "gh":
        return functools.partial(
            build_program_g, ROWS_PER_CORE, IN_ROWS, W,
            mm_dtype=mybir.dt.float16,
        )
    if variant == "g3h":
        return functools.partial(
            build_program_g3, ROWS_PER_CORE, IN_ROWS, W,
            mm_dtype=mybir.dt.float16,
        )
    if variant == "a2":
        return functools.partial(
            build_program, ROWS_A2, IN_ROWS_A2, W_A2, use_f32r=use_f32r
        )
    return functools.partial(
        build_program, ROWS_PER_CORE, IN_ROWS, W, use_f32r=use_f32r
    )


def _get_program(variant, use_f32r):
    key = (variant, use_f32r)
    if key not in _PROGRAM_CACHE:
        if variant == "g":
            _PROGRAM_CACHE[key] = build_program_g(
                ROWS_PER_CORE, IN_ROWS, W, use_f32r=use_f32r
            )
        elif variant == "gb":
            _PROGRAM_CACHE[key] = build_program_g(
                ROWS_PER_CORE, IN_ROWS, W, mm_dtype=mybir.dt.bfloat16
            )
        elif variant == "gh":
            _PROGRAM_CACHE[key] = build_program_g(
                ROWS_PER_CORE, IN_ROWS, W, mm_dtype=mybir.dt.float16
            )
        elif variant == "g3h":
            _PROGRAM_CACHE[key] = build_program_g3(
                ROWS_PER_CORE, IN_ROWS, W, mm_dtype=mybir.dt.float16
            )
        elif variant == "a2":
            _PROGRAM_CACHE[key] = build_program(
                ROWS_A2, IN_ROWS_A2, W_A2, use_f32r=use_f32r
            )
        else:
            _PROGRAM_CACHE[key] = build_program(
                ROWS_PER_CORE, IN_ROWS, W, use_f32r=use_f32r
            )
    return _PROGRAM_CACHE[key]


def _make_in_maps(x, weight, bias, variant="g"):
    x = np.ascontiguousarray(np.asarray(x, dtype=np.float32))
    b = np.asarray(bias, dtype=np.float32).reshape(1)
    if variant == "g3h":
        np_dt = np.float16
        w = np.asarray(weight, dtype=np.float32)
        w1 = w.astype(np_dt)
        w2 = (w - w1.astype(np.float32)).astype(np_dt)
        wt = np.stack(
            [build_shear_weights(w1.astype(np.float32)).astype(np_dt),
             build_shear_weights(w2.astype(np.float32)).astype(np_dt)]
        )
        in_maps = []
        for c in range(N_CORES):
            lo = c * ROWS_PER_CORE
            hi = min(H, lo + IN_ROWS)
            xb = np.zeros((IN_ROWS, W + 1), dtype=np.float32)
            xb[: hi - lo, :W] = x[lo:hi]
            xh = xb.astype(np_dt)
            xl = (xb - xh.astype(np.float32)).astype(np_dt)
            in_maps.append({"xh": xh, "xl": xl, "wt": wt, "bias": b})
        return in_maps
    if variant == "a2":
        wt = build_toeplitz(weight)
        in_maps = []
        for c in range(N_CORES):
            i, j = divmod(c, GRID_C)
            r_lo = i * (OH // GRID_R)       # 0 or 2033
            c_lo = j * COLS_A2
            xb = np.zeros((IN_ROWS_A2, W_A2), dtype=np.float32)
            r_hi = min(H, r_lo + IN_ROWS_A2)
            c_hi = min(W, c_lo + W_A2)
            xb[: r_hi - r_lo, : c_hi - c_lo] = x[r_lo:r_hi, c_lo:c_hi]
            in_maps.append({"x": xb, "wt": wt, "bias": b})
        return in_maps
    xw = W + 1 if variant in ("g", "gb", "gh") else W
    if variant in ("g", "gb", "gh"):
        wt = build_shear_weights(weight)
    else:
        wt = build_toeplitz(weight)
    np_dt = np.float32
    if variant == "gb":
        import ml_dtypes

        np_dt = ml_dtypes.bfloat16
        wt = wt.astype(np_dt)
    elif variant == "gh":
        np_dt = np.float16
        wt = wt.astype(np_dt)
    in_maps = []
    for c in range(N_CORES):
        lo = c * ROWS_PER_CORE
        hi = min(H, lo + IN_ROWS)
        xb = np.zeros((IN_ROWS, xw), dtype=np_dt)
        xb[: hi - lo, :W] = x[lo:hi].astype(np_dt)
        in_maps.append({"x": xb, "wt": wt, "bias": b})
    return in_maps


def run(x, weight, bias, variant="g", use_f32r=True, trace=False, **kwargs):
    nc = _get_program(variant, use_f32r)
    in_maps = _make_in_maps(x, weight, bias, variant)
    res = run_bass_kernel_spmd(
        nc, in_maps, core_ids=list(range(N_CORES)), trace=trace, **kwargs
    )
    if variant == "a2":
        full = np.zeros((OH, OW), dtype=np.float32)
        rpc = OH // GRID_R  # 2033
        for c in range(N_CORES):
            i, j = divmod(c, GRID_C)
            r_lo, c_lo = i * rpc, j * COLS_A2
            r_n = min(OH - r_lo, rpc)
            c_n = min(OW - c_lo, COLS_A2)
            full[r_lo : r_lo + r_n, c_lo : c_lo + c_n] = res.results[c]["out"][
                :r_n, :c_n
            ]
        return full, res
    bands = [res.results[c]["out"] for c in range(N_CORES)]
    full = np.concatenate(bands, axis=0)  # [4096, 4066]
    return full[:OH], res


def kernel(x, weight, bias):
    out, _ = run(x, weight, bias)
    return out
